# revision 15
# baseline (speedup 1.0000x reference)
"""Trainium2 Bass kernel for AlexNet-trunk + top-2 MoE (nn_Net_89343909691624).

Strategy (8 NeuronCores):
  - Data-parallel conv trunk: each core runs 8 of the 64 images through the
    AlexNet feature extractor (convs as shifted matmuls in f32r, pools as
    5D-AP max reductions on DVE).
  - AllGather of per-core features [8,9216] -> [64,9216] (channel-major).
  - Every core computes the top-2 gate for all 64 images (tiny, redundant).
  - Expert-parallel MoE: core r runs expert r's 3-layer MLP over the full
    batch, scales by its gate column, AllReduce-adds the results.

kernel(**inputs) takes the FULL unsharded inputs and returns the FULL
[64,1000] output. Everything heavy runs in ONE SPMD Bass launch.
"""
import numpy as np

from concourse import bacc, tile, mybir
from concourse import bass_utils

F32 = mybir.dt.float32
F32R = mybir.dt.float32r
AF = mybir.ActivationFunctionType
ALU = mybir.AluOpType
AX = mybir.AxisListType

NCORES = 8
IPC = 8      # images per core
PAIRS = IPC // 2


# ---------------------------------------------------------------- host prep

def _prep_static(c1w, c2w, c3w, c4w, c5w, gw1, gw2, gb2,
                 c1b, c2b, c3b, c4b, c5b, gb1):
    """Core-independent weight rearrangement (done once, shared by all cores)."""
    o = {}
    # conv1: row-phase fold K=(dy,KH,c)=36, kw kept as 11 strided taps
    c1p = np.zeros((64, 3, 12, 11), np.float32)
    c1p[:, :, :11, :] = c1w
    # [o, c, KH, dy, kw] -> [dy, KH, c, kw, o]
    t = c1p.reshape(64, 3, 3, 4, 11).transpose(3, 2, 1, 4, 0)
    o["w1n"] = np.ascontiguousarray(t.reshape(36, 11, 64))
    # conv2: kh-pair folding. K=(d,c): p = d*64+c
    t = c2w.transpose(1, 2, 3, 0)  # [c64, kh5, kw5, o192]
    o["w2a"] = np.ascontiguousarray(
        np.concatenate([t[:, 0:4:2], t[:, 1:5:2]], axis=0))  # [128, khp2, kw5, 192]
    o["w2b"] = np.ascontiguousarray(t[:, 4])                 # [64, kw5, 192]
    t = c3w.transpose(1, 2, 3, 0).reshape(192, 9, 384)
    o["w3a"] = np.ascontiguousarray(t[:128])
    o["w3b"] = np.ascontiguousarray(t[128:])
    t = c4w.transpose(1, 2, 3, 0).reshape(384, 9, 256)
    o["w4t"] = np.ascontiguousarray(t.reshape(3, 128, 9, 256).transpose(1, 0, 2, 3))
    t = c5w.transpose(1, 2, 3, 0).reshape(256, 9, 256)
    o["w5t"] = np.ascontiguousarray(t.reshape(2, 128, 9, 256).transpose(1, 0, 2, 3))
    # gate
    o["gw1r"] = np.ascontiguousarray(
        gw1.reshape(2, 128, 36, 72).transpose(1, 0, 2, 3))   # [128, kc2, s36, 72]
    o["gw2b"] = np.ascontiguousarray(
        np.concatenate([gw2, gb2[None, :]], axis=0))          # [73, 8]
    # biases (packed per-partition, per M-chunk)
    o["c1bp"] = c1b.reshape(64, 1).astype(np.float32)
    b = np.zeros((128, 2), np.float32)
    b[:, 0] = c2b[:128]
    b[:64, 1] = c2b[128:]
    o["c2bp"] = b
    o["c3bp"] = np.ascontiguousarray(c3b.reshape(3, 128).T)
    o["c4bp"] = np.ascontiguousarray(c4b.reshape(2, 128).T)
    o["c5bp"] = np.ascontiguousarray(c5b.reshape(2, 128).T)
    o["gb1p"] = gb1.reshape(72, 1).astype(np.float32)
    o["ident"] = np.eye(64, dtype=np.float32)
    o["zeros"] = np.zeros((128, 6498), np.float32)
    o["ones64"] = np.ones((1, 64), np.float32)
    return o


def _prep_expert(ew1, eb1, ew2, eb2, ew3, eb3, r):
    """Expert-r weight rearrangement."""
    o = {}
    o["ew1r"] = np.ascontiguousarray(
        ew1[r].reshape(2, 128, 36, 288).transpose(1, 0, 2, 3))  # [128, kc2, s36, 288]
    o["eb1p"] = np.ascontiguousarray(eb1[r].reshape(3, 96).T)   # [96, 3]
    e2 = ew2[r]
    o["e2r"] = np.ascontiguousarray(
        np.concatenate([e2[0:96], eb2[r][None, :], e2[96:192], e2[192:288]], axis=0))
    e3 = ew3[r]
    o["e3r"] = np.ascontiguousarray(
        np.concatenate([e3[0:72], eb3[r][None, :], e3[72:144]], axis=0))  # [145, 1000]
    esel = np.zeros((64, 8), np.float32)
    esel[:, r] = 1.0
    o["esel"] = esel
    return o


# ---------------------------------------------------------------- AP helpers

def win3s2(ap, Ho, Wo, rowstep):
    """Overlapping 3x3/stride-2 window view [C, Ho, Wo, 3, 3] of a [C, H, W] AP."""
    w = ap.copy()
    part = w.ap[0]
    w.ap = mybir.VecI64Pair(
        [list(part), [2 * rowstep, Ho], [2, Wo], [rowstep, 3], [1, 3]])
    return w


# ---------------------------------------------------------------- builder

def build_nc():
    nc = bacc.Bacc("TRN2", target_bir_lowering=False, debug=False,
                   num_devices=NCORES)

    def inp(name, shape, dt=F32R):
        return nc.dram_tensor(name, list(shape), dt, kind="ExternalInput").ap()

    xs = inp("xs", [IPC, 3, 224, 224])
    w1n = inp("w1n", [36, 11, 64])
    w2a = inp("w2a", [128, 2, 5, 192])
    w2b = inp("w2b", [64, 5, 192])
    w3a = inp("w3a", [128, 9, 384])
    w3b = inp("w3b", [64, 9, 384])
    w4t = inp("w4t", [128, 3, 9, 256])
    w5t = inp("w5t", [128, 2, 9, 256])
    gw1r = inp("gw1r", [128, 2, 36, 72])
    gw2b = inp("gw2b", [73, 8])
    ew1r = inp("ew1r", [128, 2, 36, 288])
    e2r = inp("e2r", [289, 144])
    e3r = inp("e3r", [145, 1000])
    c1bp = inp("c1bp", [64, 1], F32)
    c2bp = inp("c2bp", [128, 2], F32)
    c3bp = inp("c3bp", [128, 3], F32)
    c4bp = inp("c4bp", [128, 2], F32)
    c5bp = inp("c5bp", [128, 2], F32)
    gb1p = inp("gb1p", [72, 1], F32)
    eb1p = inp("eb1p", [96, 3], F32)
    esel = inp("esel", [64, 8], F32)
    ident = inp("ident", [64, 64], F32)
    zeros = inp("zeros", [128, 6498])
    ones64 = inp("ones64", [1, 64])

    out = nc.dram_tensor("out", [64, 1000], F32, kind="ExternalOutput").ap()

    with tile.TileContext(nc) as tc:
        _build_body(nc, tc, locals())
    nc.compile()
    return nc


def _build_body(nc, tc, io):
    xs = io["xs"]

    with (
        tc.tile_pool(name="wts", bufs=1) as wts,
        tc.tile_pool(name="acts", bufs=1) as acts,
        tc.tile_pool(name="pool2", bufs=1) as pl2,
        tc.tile_pool(name="gwp", bufs=8) as gwp,
        tc.tile_pool(name="ew1p", bufs=8) as ew1p,
        tc.tile_pool(name="w45p", bufs=6) as w45p,
        tc.tile_pool(name="x36p", bufs=2) as x36p,
        tc.tile_pool(name="dram", bufs=1, space="DRAM") as dram,
    ):
        # ---- resident weights
        W1 = wts.tile([36, 11, 64], F32R, tag="W1")
        W2A = wts.tile([128, 2, 5, 192], F32R, tag="W2A")
        W2B = wts.tile([64, 5, 192], F32R, tag="W2B")
        W3A = wts.tile([128, 9, 384], F32R, tag="W3A")
        W3B = wts.tile([64, 9, 384], F32R, tag="W3B")
        GW2B = wts.tile([73, 8], F32R, tag="GW2B")
        E2_0 = wts.tile([97, 144], F32R, tag="E2_0")
        E2_1 = wts.tile([96, 144], F32R, tag="E2_1")
        E2_2 = wts.tile([96, 144], F32R, tag="E2_2")
        E3_0 = wts.tile([73, 1000], F32R, tag="E3_0")
        E3_1 = wts.tile([72, 1000], F32R, tag="E3_1")
        IDENT = wts.tile([64, 64], F32, tag="IDENT")
        C1B = wts.tile([64, 1], F32, tag="C1B")
        C2B = wts.tile([128, 2], F32, tag="C2B")
        C3B = wts.tile([128, 3], F32, tag="C3B")
        C4B = wts.tile([128, 2], F32, tag="C4B")
        C5B = wts.tile([128, 2], F32, tag="C5B")
        GB1 = wts.tile([72, 1], F32, tag="GB1")
        EB1 = wts.tile([96, 3], F32, tag="EB1")
        ESEL = wts.tile([64, 8], F32, tag="ESEL")

        for t, a in [(W1, io["w1n"]), (W2A, io["w2a"]), (W2B, io["w2b"]),
                     (W3A, io["w3a"]), (W3B, io["w3b"]),
                     (GW2B, io["gw2b"]), (IDENT, io["ident"]),
                     (C1B, io["c1bp"]), (C2B, io["c2bp"]), (C3B, io["c3bp"]),
                     (C4B, io["c4bp"]), (C5B, io["c5bp"]), (GB1, io["gb1p"]),
                     (EB1, io["eb1p"]), (ESEL, io["esel"])]:
            nc.sync.dma_start(t[:], a)
        nc.sync.dma_start(E2_0[:], io["e2r"][0:97, :])
        nc.sync.dma_start(E2_1[:], io["e2r"][97:193, :])
        nc.sync.dma_start(E2_2[:], io["e2r"][193:289, :])
        nc.sync.dma_start(E3_0[:], io["e3r"][0:73, :])
        nc.sync.dma_start(E3_1[:], io["e3r"][73:145, :])

        # ---- persistent padded activation tiles (borders stay zero)
        P1D = [acts.tile([128, 2, 31, 32], F32R, tag=f"P1D{k}", name=f"P1D{k}")
               for k in range(1)]
        P2 = [(acts.tile([128, 2, 15, 16], F32R, tag=f"P2a{k}", name=f"P2a{k}"),
               acts.tile([64, 2, 15, 16], F32R, tag=f"P2b{k}", name=f"P2b{k}"))
              for k in range(2)]
        P3 = [[acts.tile([128, 2, 15, 16], F32R, tag=f"P3_{k}_{m}", name=f"P3_{k}_{m}")
               for m in range(3)] for k in range(2)]
        P4 = [[acts.tile([128, 2, 15, 16], F32R, tag=f"P4_{k}_{m}", name=f"P4_{k}_{m}")
               for m in range(2)] for k in range(2)]
        def zfill(t):
            f = t[:].rearrange("c a h w -> c (a h w)")
            nc.sync.dma_start(f, io["zeros"][0:f.shape[0], 0:f.shape[1]])
        for t in P1D:
            zfill(t)
        for k in range(2):
            zfill(P2[k][0])
            zfill(P2[k][1])
            for m in range(3):
                zfill(P3[k][m])
            for m in range(2):
                zfill(P4[k][m])

        # DRAM bounce buffers for collectives
        feat_loc = dram.tile([IPC, 256, 36], F32R)
        feat_all = dram.tile([64, 256, 36], F32R, addr_space="Shared")
        ar_in = dram.tile([64, 1000], F32)
        ar_out = dram.tile([64, 1000], F32, addr_space="Shared")

        with (
            tc.tile_pool(name="c1ps", bufs=2, space="PSUM") as c1ps,
            tc.tile_pool(name="c2ps", bufs=1, space="PSUM") as c2ps,
            tc.tile_pool(name="c345ps", bufs=4, space="PSUM") as c345ps,
        ):
            for p in range(PAIRS):
                _conv_trunk_pair(nc, tc, io, p, xs, x36p, P1D, P2, P3, P4,
                                 W1, W2A, W2B, W3A, W3B, w45p,
                                 C1B, C2B, C3B, C4B, C5B,
                                 acts, pl2, c1ps, c2ps, c345ps, feat_loc)

        # ---- AllGather features
        nc.gpsimd.collective_compute(
            "AllGather", ALU.bypass,
            replica_groups=[list(range(NCORES))],
            ins=[feat_loc[:].opt()],
            outs=[feat_all[:].opt()],
        )

        with tc.tile_pool(name="mps", bufs=1, space="PSUM") as mps:
            _moe(nc, tc, io, feat_all, gwp, ew1p, wts, acts, mps,
                 GW2B, GB1, EB1, E2_0, E2_1, E2_2, E3_0, E3_1, IDENT, ESEL,
                 ar_in)

        # ---- AllReduce the gate-weighted expert outputs
        nc.gpsimd.collective_compute(
            "AllReduce", ALU.add,
            replica_groups=[list(range(NCORES))],
            ins=[ar_in[:].opt()],
            outs=[ar_out[:].opt()],
        )
        nc.sync.dma_start(io["out"], ar_out[:])


def _conv_trunk_pair(nc, tc, io, p, xs, x36p, P1D, P2, P3, P4,
                     W1, W2A, W2B, W3A, W3B, w45p,
                     C1B, C2B, C3B, C4B, C5B,
                     acts, pl2, c1ps, c2ps, c345ps, feat_loc):
    k = p % 2  # parity for double-buffered padded tiles
    P1d = P1D[0]
    P2a, P2b = P2[k]

    # ---- conv1: X36[(dy,KH,c), y', v] = xpad[c, 4*(y0+y'+KH)+dy-2, v] loaded
    # per row-chunk (contiguous-row DMAs); 11 kw taps read with stride-4 APs.
    row_chunks = [(0, 8), (8, 8), (16, 8), (24, 8), (32, 8), (40, 8), (48, 7)]
    for j in range(2):
        A1 = acts.tile([64, 55, 55], F32R, tag="A1", name="A1")
        for ci, (y0, rows) in enumerate(row_chunks):
            xt = x36p.tile([36, 8, 232], F32R, tag="X36", name="X36")
            if ci in (0, 1, 6):
                f = xt[:].rearrange("c a v -> c (a v)")
                nc.sync.dma_start(f, io["zeros"][0:36, 0:f.shape[1]])
            for dy in range(4):
                for KH in range(3):
                    pb = (dy * 3 + KH) * 3
                    ylo = 0
                    while 4 * (y0 + ylo + KH) + dy - 2 < 0:
                        ylo += 1
                    yhi = rows
                    while yhi > ylo and 4 * (y0 + yhi - 1 + KH) + dy - 2 > 223:
                        yhi -= 1
                    if yhi <= ylo:
                        continue
                    r0 = 4 * (y0 + ylo + KH) + dy - 2
                    nstep = yhi - ylo
                    nc.sync.dma_start(
                        xt[pb:pb + 3, ylo:yhi, 2:226],
                        xs[2 * p + j, :, r0:r0 + 4 * (nstep - 1) + 1:4, :])
            ps = c1ps.tile([64, 8, 56], F32, tag="c1")
            for kw in range(11):
                nc.tensor.matmul(
                    ps[:, 0:rows, :],
                    W1[:, kw, :],
                    xt[:, 0:rows, kw:kw + 221:4],
                    start=(kw == 0), stop=(kw == 10))
            nc.scalar.activation(A1[:, y0:y0 + rows, :], ps[:, 0:rows, 0:55],
                                 AF.Relu, bias=C1B[:])
        # pool1: 55x55 -> 27x27 into P1d interior (pad=2)
        nc.vector.tensor_reduce(
            out=P1d[0:64, j, 2:29, 2:29],
            in_=win3s2(A1[:].bitcast(F32), 27, 27, 55),
            axis=AX.XY, op=ALU.max)
    # replicate row-shifted copy into upper partitions: P1d[64+c, j, h] = P1d[c, j, h+1]
    for j in range(2):
        nc.sync.dma_start(P1D[0][64:128, j, 0:30, :], P1D[0][0:64, j, 1:31, :])

    # ---- conv2 (kh-folded: khp0/1 K=128, khp2 K=64; 5 kw; M=192) -> A2, pool2 -> P2
    for j in range(2):
        A2a = acts.tile([128, 27, 27], F32R, tag="A2a", name="A2a")
        A2b = acts.tile([64, 27, 27], F32R, tag="A2b", name="A2b")
        for rc, (y0, rows) in enumerate([(0, 14), (14, 13)]):
            for mc, (msz, A2, ps_tag) in enumerate(
                    [(128, A2a, "c2a"), (64, A2b, "c2b")]):
                mlo = mc * 128
                ps = c2ps.tile([msz, 14, 28], F32, tag=ps_tag)
                first = True
                for khp in range(2):
                    for kw in range(5):
                        nc.tensor.matmul(
                            ps[:, 0:rows, :],
                            W2A[:, khp, kw, mlo:mlo + msz],
                            P1d[:, j, y0 + 2 * khp:y0 + 2 * khp + rows, kw:kw + 28],
                            start=first, stop=False)
                        first = False
                for kw in range(5):
                    nc.tensor.matmul(
                        ps[:, 0:rows, :],
                        W2B[:, kw, mlo:mlo + msz],
                        P1d[0:64, j, y0 + 4:y0 + 4 + rows, kw:kw + 28],
                        start=False, stop=(kw == 4))
                nc.scalar.activation(A2[:, y0:y0 + rows, :], ps[:, 0:rows, 0:27],
                                     AF.Relu, bias=C2B[0:msz, mc:mc + 1])
        # pool2: 27x27 -> 13x13 into P2 interior (pad=1)
        nc.vector.tensor_reduce(
            out=P2a[:, j, 1:14, 1:14],
            in_=win3s2(A2a[:].bitcast(F32), 13, 13, 27),
            axis=AX.XY, op=ALU.max)
        nc.vector.tensor_reduce(
            out=P2b[:, j, 1:14, 1:14],
            in_=win3s2(A2b[:].bitcast(F32), 13, 13, 27),
            axis=AX.XY, op=ALU.max)

    k2 = p % 2
    P3k, P4k = P3[k2], P4[k2]

    # ---- conv3: K=192 (128+64), M=384 (3x128), both images at once
    for mc in range(3):
        ps = c345ps.tile([128, 2, 13, 14], F32, tag="c345")
        first = True
        for kh in range(3):
            for kw in range(3):
                nc.tensor.matmul(
                    ps[:], W3A[:, kh * 3 + kw, mc * 128:(mc + 1) * 128],
                    P2a[:, :, kh:kh + 13, kw:kw + 14],
                    start=first, stop=False)
                first = False
                nc.tensor.matmul(
                    ps[:], W3B[:, kh * 3 + kw, mc * 128:(mc + 1) * 128],
                    P2b[:, :, kh:kh + 13, kw:kw + 14],
                    start=False, stop=(kh == 2 and kw == 2))
        nc.scalar.activation(P3k[mc][:, :, 1:14, 1:14], ps[:, :, :, 0:13],
                             AF.Relu, bias=C3B[:, mc:mc + 1])

    # ---- conv4: K=384 (3x128), M=256 (2x128), weights streamed per tap
    ps4 = [c345ps.tile([128, 2, 13, 14], F32, tag="c345", name=f"c4ps{m}")
           for m in range(2)]
    first = True
    for kh in range(3):
        for kw in range(3):
            for kc in range(3):
                w4s = w45p.tile([128, 256], F32R, tag="w45s", name="w4s")
                nc.sync.dma_start(w4s[:], io["w4t"][:, kc, kh * 3 + kw, :])
                for mc in range(2):
                    nc.tensor.matmul(
                        ps4[mc][:], w4s[:, mc * 128:(mc + 1) * 128],
                        P3k[kc][:, :, kh:kh + 13, kw:kw + 14],
                        start=first, stop=(kh == 2 and kw == 2 and kc == 2))
                first = False
    for mc in range(2):
        nc.scalar.activation(P4k[mc][:, :, 1:14, 1:14], ps4[mc][:, :, :, 0:13],
                             AF.Relu, bias=C4B[:, mc:mc + 1])

    # ---- conv5: K=256 (2x128), M=256 (2x128), streamed weights -> A5, pool3 -> F
    ps5 = [c345ps.tile([128, 2, 13, 14], F32, tag="c345", name=f"c5ps{m}")
           for m in range(2)]
    first = True
    for kh in range(3):
        for kw in range(3):
            for kc in range(2):
                w5s = w45p.tile([128, 256], F32R, tag="w45s", name="w5s")
                nc.sync.dma_start(w5s[:], io["w5t"][:, kc, kh * 3 + kw, :])
                for mc in range(2):
                    nc.tensor.matmul(
                        ps5[mc][:], w5s[:, mc * 128:(mc + 1) * 128],
                        P4k[kc][:, :, kh:kh + 13, kw:kw + 14],
                        start=first, stop=(kh == 2 and kw == 2 and kc == 1))
                first = False
    for mc in range(2):
        A5 = pl2.tile([128, 2, 13, 13], F32R, tag=f"A5_{mc}", name=f"A5_{mc}")
        nc.scalar.activation(A5[:], ps5[mc][:, :, :, 0:13], AF.Relu,
                             bias=C5B[:, mc:mc + 1])
        F = pl2.tile([128, 2, 6, 6], F32R, tag=f"F_{mc}", name=f"F_{mc}")
        for j in range(2):
            nc.vector.tensor_reduce(
                out=F[:, j],
                in_=win3s2(A5[:, j].bitcast(F32), 6, 6, 13),
                axis=AX.XY, op=ALU.max)
        # store features channel-major: feat_loc[b, c, s]
        nc.sync.dma_start(
            feat_loc[:].rearrange("b c s -> c b s")
            [mc * 128:(mc + 1) * 128, 2 * p:2 * p + 2, :],
            F[:].rearrange("c j x y -> c j (x y)"))


def _moe(nc, tc, io, feat_all, gwp, ew1p, wts, acts, mps,
         GW2B, GB1, EB1, E2_0, E2_1, E2_2, E3_0, E3_1, IDENT, ESEL, ar_in):
    # load gathered features channel-major: FA[kc] = [128, 64img, 36]
    FA = [acts.tile([128, 64, 36], F32R, tag=f"FA{kc}", name=f"FA{kc}")
          for kc in range(2)]
    for kc in range(2):
        nc.sync.dma_start(
            FA[kc][:],
            feat_all[:].rearrange("b c s -> c b s")[kc * 128:(kc + 1) * 128, :, :])

    # ---- gate: g = relu(feat @ gw1 + gb1) (g kept transposed [72, 64])
    gps = mps.tile([72, 64], F32, tag="gps")
    first = True
    for kc in range(2):
        for s in range(36):
            gt = gwp.tile([128, 72], F32R, tag="gw1")
            nc.sync.dma_start(gt[:], io["gw1r"][:, kc, s, :])
            nc.tensor.matmul(gps[:], gt[:], FA[kc][:, :, s],
                             start=first, stop=(kc == 1 and s == 35))
            first = False
    G1 = acts.tile([73, 64], F32R, tag="G1")
    nc.scalar.activation(G1[0:72, :], gps[:], AF.Relu, bias=GB1[:])
    nc.sync.dma_start(G1[72:73, :], io["ones64"])
    # logits[64, 8] = G1.T @ [gw2; gb2]
    lps = mps.tile([64, 8], F32, tag="lps")
    nc.tensor.matmul(lps[:], G1[:], GW2B[:], start=True, stop=True)
    L = acts.tile([64, 8], F32, tag="L")
    nc.scalar.activation(L[:], lps[:], AF.Copy)

    # ---- top-2 softmax -> this core's gate column [64, 1]
    v1 = acts.tile([64, 1], F32, tag="v1")
    nc.vector.tensor_reduce(out=v1[:], in_=L[:], axis=AX.X, op=ALU.max)
    nv1 = acts.tile([64, 1], F32, tag="nv1")
    nc.vector.tensor_scalar_mul(nv1[:], v1[:], -1.0)
    m1 = acts.tile([64, 8], F32, tag="m1")
    nc.vector.tensor_scalar(out=m1[:], in0=L[:], scalar1=v1[:], scalar2=None,
                            op0=ALU.is_equal)
    lm = acts.tile([64, 8], F32, tag="lm")
    nc.vector.scalar_tensor_tensor(out=lm[:], in0=m1[:], scalar=-1e30,
                                   in1=L[:], op0=ALU.mult, op1=ALU.add)
    v2 = acts.tile([64, 1], F32, tag="v2")
    nc.vector.tensor_reduce(out=v2[:], in_=lm[:], axis=AX.X, op=ALU.max)
    e2v = acts.tile([64, 1], F32, tag="e2v")
    nc.scalar.activation(e2v[:], v2[:], AF.Exp, bias=nv1[:])
    den = acts.tile([64, 1], F32, tag="den")
    nc.vector.tensor_scalar_add(den[:], e2v[:], 1.0)
    inv = acts.tile([64, 1], F32, tag="inv")
    nc.vector.reciprocal(inv[:], den[:])
    expl = acts.tile([64, 8], F32, tag="expl")
    nc.scalar.activation(expl[:], L[:], AF.Exp, bias=nv1[:])
    msk = acts.tile([64, 8], F32, tag="msk")
    nc.vector.tensor_scalar(out=msk[:], in0=L[:], scalar1=v2[:], scalar2=None,
                            op0=ALU.is_ge)
    t1 = acts.tile([64, 8], F32, tag="t1")
    nc.vector.tensor_tensor(out=t1[:], in0=msk[:], in1=expl[:], op=ALU.mult)
    t2 = acts.tile([64, 8], F32, tag="t2")
    nc.vector.tensor_tensor(out=t2[:], in0=t1[:], in1=ESEL[:], op=ALU.mult)
    gs = acts.tile([64, 1], F32, tag="gs")
    nc.vector.tensor_reduce(out=gs[:], in_=t2[:], axis=AX.X, op=ALU.add)
    gate = acts.tile([64, 1], F32, tag="gate")
    nc.vector.tensor_scalar(out=gate[:], in0=gs[:], scalar1=inv[:], scalar2=None,
                            op0=ALU.mult)

    # ---- FC1: h1T[mc] = relu(sum_{kc,s} ew1[kc,s].T @ featT + eb1) ([96, 64] x3)
    h1ps = [mps.tile([96, 64], F32, tag=f"h1ps{m}", name=f"h1ps{m}")
            for m in range(3)]
    first = True
    for kc in range(2):
        for s in range(36):
            et = ew1p.tile([128, 288], F32R, tag="ew1")
            nc.sync.dma_start(et[:], io["ew1r"][:, kc, s, :])
            for m in range(3):
                nc.tensor.matmul(h1ps[m][:], et[:, m * 96:(m + 1) * 96],
                                 FA[kc][:, :, s],
                                 start=first, stop=(kc == 1 and s == 35))
            first = False
    H1 = [acts.tile([97 if m == 0 else 96, 64], F32R, tag=f"H1_{m}", name=f"H1_{m}")
          for m in range(3)]
    for m in range(3):
        nc.scalar.activation(H1[m][0:96, :], h1ps[m][:], AF.Relu,
                             bias=EB1[:, m:m + 1])
    nc.sync.dma_start(H1[0][96:97, :], io["ones64"])

    # ---- FC2: h2[64, 144] = relu(h1 @ ew2 + eb2)
    h2ps = mps.tile([64, 144], F32, tag="h2ps")
    nc.tensor.matmul(h2ps[:], H1[0][:], E2_0[:], start=True, stop=False)
    nc.tensor.matmul(h2ps[:], H1[1][:], E2_1[:], start=False, stop=False)
    nc.tensor.matmul(h2ps[:], H1[2][:], E2_2[:], start=False, stop=True)
    H2 = acts.tile([64, 144], F32, tag="H2")
    nc.scalar.activation(H2[:], h2ps[:], AF.Relu)

    # transpose h2 -> [72, 64] x2 (PE transpose via identity)
    H2T = [acts.tile([73 if c == 0 else 72, 64], F32R, tag=f"H2T{c}", name=f"H2T{c}")
           for c in range(2)]
    for c in range(2):
        tps = mps.tile([72, 64], F32, tag="tps")
        nc.tensor.transpose(tps[:], H2[:, c * 72:(c + 1) * 72], IDENT[:])
        nc.scalar.activation(H2T[c][0:72, :], tps[:], AF.Copy)
    nc.sync.dma_start(H2T[0][72:73, :], io["ones64"])

    # ---- FC3 + gate scale: eo[64, 1000] = (h2 @ ew3 + eb3) * gate
    for nchunk in range(2):
        nlo = nchunk * 500
        eps = mps.tile([64, 500], F32, tag="eps", name="eps")
        nc.tensor.matmul(eps[:], H2T[0][:], E3_0[:, nlo:nlo + 500],
                         start=True, stop=False)
        nc.tensor.matmul(eps[:], H2T[1][:], E3_1[:, nlo:nlo + 500],
                         start=False, stop=True)
        eps_s = acts.tile([64, 500], F32, tag="eps_s", name="eps_s")
        nc.scalar.activation(eps_s[:], eps[:], AF.Copy, scale=gate[:])
        nc.sync.dma_start(ar_in[:, nlo:nlo + 500], eps_s[:])


# ---------------------------------------------------------------- runner

_CACHE = {}


def _get_state():
    if "nc" not in _CACHE:
        _CACHE["nc"] = build_nc()
    return _CACHE["nc"]


def _make_in_maps(inputs):
    static = _prep_static(
        inputs["c1w"], inputs["c2w"], inputs["c3w"], inputs["c4w"],
        inputs["c5w"], inputs["gw1"], inputs["gw2"], inputs["gb2"],
        inputs["c1b"], inputs["c2b"], inputs["c3b"], inputs["c4b"],
        inputs["c5b"], inputs["gb1"])
    x = np.asarray(inputs["x"], np.float32)
    in_maps = []
    for r in range(NCORES):
        m = dict(static)
        m.update(_prep_expert(inputs["ew1"], inputs["eb1"], inputs["ew2"],
                              inputs["eb2"], inputs["ew3"], inputs["eb3"], r))
        m["xs"] = np.ascontiguousarray(x[IPC * r:IPC * (r + 1)])
        in_maps.append({k: np.asarray(v) for k, v in m.items()})
    return in_maps


def kernel(**inputs):
    inputs = {k: np.asarray(v) for k, v in inputs.items()}
    nc = _get_state()
    in_maps = _make_in_maps(inputs)
    res = bass_utils.run_bass_kernel_spmd(nc, in_maps,
                                          core_ids=list(range(NCORES)))
    return res.results[0]["out"].astype(np.float32)


# revision 22
# speedup vs baseline: 77.4736x; 77.4736x over previous
"""Trainium2 Bass kernel for AlexNet-trunk + top-2 MoE (nn_Net_89343909691624).

Strategy (8 NeuronCores):
  - Data-parallel conv trunk: each core runs 8 of the 64 images through the
    AlexNet feature extractor (convs as shifted matmuls in f32r, pools as
    5D-AP max reductions on DVE).
  - AllGather of per-core features [8,9216] -> [64,9216] (channel-major).
  - Every core computes the top-2 gate for all 64 images (tiny, redundant).
  - Expert-parallel MoE: core r runs expert r's 3-layer MLP over the full
    batch, scales by its gate column, AllReduce-adds the results.

kernel(**inputs) takes the FULL unsharded inputs and returns the FULL
[64,1000] output. Everything heavy runs in ONE SPMD Bass launch.
"""
import os

import numpy as np

os.environ.setdefault("JAX_COMPILATION_CACHE_DIR", "/tmp/jaxcache")
try:
    import jax as _jax
    _jax.config.update("jax_compilation_cache_dir", "/tmp/jaxcache")
    _jax.config.update("jax_persistent_cache_min_entry_size_bytes", 0)
    _jax.config.update("jax_persistent_cache_min_compile_time_secs", 0.0)
except Exception:
    pass

from concourse import bacc, tile, mybir
from concourse import bass_utils

F32 = mybir.dt.float32
F32R = mybir.dt.float32r
AF = mybir.ActivationFunctionType
ALU = mybir.AluOpType
AX = mybir.AxisListType

NCORES = 8
IPC = 8      # images per core
PAIRS = IPC // 2


# ---------------------------------------------------------------- host prep

def _prep_static(c1w, c2w, c3w, c4w, c5w, gw1, gw2, gb2,
                 c1b, c2b, c3b, c4b, c5b, gb1):
    """Core-independent weight rearrangement (done once, shared by all cores)."""
    o = {}
    # conv1: row-phase fold K=(dy,KH,c)=36, kw kept as 11 strided taps
    c1p = np.zeros((64, 3, 12, 11), np.float32)
    c1p[:, :, :11, :] = c1w
    # [o, c, KH, dy, kw] -> [dy, KH, c, kw, o]
    t = c1p.reshape(64, 3, 3, 4, 11).transpose(3, 2, 1, 4, 0)
    o["w1n"] = np.ascontiguousarray(t.reshape(36, 11, 64))
    # conv2: kh-pair folding. K=(d,c): p = d*64+c
    t = c2w.transpose(1, 2, 3, 0)  # [c64, kh5, kw5, o192]
    o["w2a"] = np.ascontiguousarray(
        np.concatenate([t[:, 0:4:2], t[:, 1:5:2]], axis=0))  # [128, khp2, kw5, 192]
    o["w2b"] = np.ascontiguousarray(t[:, 4])                 # [64, kw5, 192]
    t = c3w.transpose(1, 2, 3, 0).reshape(192, 9, 384)
    o["w3a"] = np.ascontiguousarray(t[:128])
    o["w3b"] = np.ascontiguousarray(t[128:])
    t = c4w.transpose(1, 2, 3, 0).reshape(384, 9, 256)
    o["w4t"] = np.ascontiguousarray(t.reshape(3, 128, 9, 256).transpose(1, 0, 2, 3))
    t = c5w.transpose(1, 2, 3, 0).reshape(256, 9, 256)
    o["w5t"] = np.ascontiguousarray(t.reshape(2, 128, 9, 256).transpose(1, 0, 2, 3))
    # gate
    o["gw1r"] = np.ascontiguousarray(
        gw1.reshape(2, 128, 36, 72).transpose(1, 0, 2, 3))   # [128, kc2, s36, 72]
    o["gw2b"] = np.ascontiguousarray(
        np.concatenate([gw2, gb2[None, :]], axis=0))          # [73, 8]
    # biases (packed per-partition, per M-chunk)
    o["c1bp"] = c1b.reshape(64, 1).astype(np.float32)
    b = np.zeros((128, 2), np.float32)
    b[:, 0] = c2b[:128]
    b[:64, 1] = c2b[128:]
    o["c2bp"] = b
    o["c3bp"] = np.ascontiguousarray(c3b.reshape(3, 128).T)
    o["c4bp"] = np.ascontiguousarray(c4b.reshape(2, 128).T)
    o["c5bp"] = np.ascontiguousarray(c5b.reshape(2, 128).T)
    o["gb1p"] = gb1.reshape(72, 1).astype(np.float32)
    o["ident"] = np.eye(64, dtype=np.float32)
    o["zeros"] = np.zeros((128, 6498), np.float32)
    o["ones64"] = np.ones((1, 64), np.float32)
    return o


def _prep_expert(ew1, eb1, ew2, eb2, ew3, eb3, r):
    """Expert-r weight rearrangement."""
    o = {}
    o["ew1r"] = np.ascontiguousarray(
        ew1[r].reshape(2, 128, 36, 288).transpose(1, 0, 2, 3))  # [128, kc2, s36, 288]
    o["eb1p"] = np.ascontiguousarray(eb1[r].reshape(3, 96).T)   # [96, 3]
    e2 = ew2[r]
    o["e2r"] = np.ascontiguousarray(
        np.concatenate([e2[0:96], eb2[r][None, :], e2[96:192], e2[192:288]], axis=0))
    e3 = ew3[r]
    o["e3r"] = np.ascontiguousarray(
        np.concatenate([e3[0:72], eb3[r][None, :], e3[72:144]], axis=0))  # [145, 1000]
    esel = np.zeros((64, 8), np.float32)
    esel[:, r] = 1.0
    o["esel"] = esel
    return o


# ---------------------------------------------------------------- AP helpers

def win3s2(ap, Ho, Wo, rowstep):
    """Overlapping 3x3/stride-2 window view [C, Ho, Wo, 3, 3] of a [C, H, W] AP."""
    w = ap.copy()
    part = w.ap[0]
    w.ap = mybir.VecI64Pair(
        [list(part), [2 * rowstep, Ho], [2, Wo], [rowstep, 3], [1, 3]])
    return w


# ---------------------------------------------------------------- builder

def build_nc(single=False):
    nc = bacc.Bacc("TRN2", target_bir_lowering=False, debug=False,
                   num_devices=1 if single else NCORES)

    def inp(name, shape, dt=F32R):
        return nc.dram_tensor(name, list(shape), dt, kind="ExternalInput").ap()

    xs = inp("xs", [IPC, 3, 224, 224])
    w1n = inp("w1n", [36, 11, 64])
    w2a = inp("w2a", [128, 2, 5, 192])
    w2b = inp("w2b", [64, 5, 192])
    w3a = inp("w3a", [128, 9, 384])
    w3b = inp("w3b", [64, 9, 384])
    w4t = inp("w4t", [128, 3, 9, 256])
    w5t = inp("w5t", [128, 2, 9, 256])
    gw1r = inp("gw1r", [128, 2, 36, 72])
    gw2b = inp("gw2b", [73, 8])
    ew1r = inp("ew1r", [128, 2, 36, 288])
    e2r = inp("e2r", [289, 144])
    e3r = inp("e3r", [145, 1000])
    c1bp = inp("c1bp", [64, 1], F32)
    c2bp = inp("c2bp", [128, 2], F32)
    c3bp = inp("c3bp", [128, 3], F32)
    c4bp = inp("c4bp", [128, 2], F32)
    c5bp = inp("c5bp", [128, 2], F32)
    gb1p = inp("gb1p", [72, 1], F32)
    eb1p = inp("eb1p", [96, 3], F32)
    esel = inp("esel", [64, 8], F32)
    ident = inp("ident", [64, 64], F32)
    zeros = inp("zeros", [128, 6498])
    ones64 = inp("ones64", [1, 64])

    out = nc.dram_tensor("out", [64, 1000], F32, kind="ExternalOutput").ap()

    with tile.TileContext(nc) as tc:
        _build_body(nc, tc, locals(), single=single)
    nc.compile()
    return nc


def _build_body(nc, tc, io, single=False):
    xs = io["xs"]

    with (
        tc.tile_pool(name="wts", bufs=1) as wts,
        tc.tile_pool(name="acts", bufs=1) as acts,
        tc.tile_pool(name="pool2", bufs=1) as pl2,
        tc.tile_pool(name="gwp", bufs=2) as gwp,
        tc.tile_pool(name="ew1p", bufs=2) as ew1p,
        tc.tile_pool(name="w45p", bufs=2) as w45p,
        tc.tile_pool(name="x36p", bufs=2) as x36p,
        tc.tile_pool(name="dram", bufs=1, space="DRAM") as dram,
    ):
        # ---- resident weights
        W1 = wts.tile([36, 11, 64], F32R, tag="W1")
        W2A = wts.tile([128, 2, 5, 192], F32R, tag="W2A")
        W2B = wts.tile([64, 5, 192], F32R, tag="W2B")
        W3A = wts.tile([128, 9, 384], F32R, tag="W3A")
        W3B = wts.tile([64, 9, 384], F32R, tag="W3B")
        GW2B = wts.tile([73, 8], F32R, tag="GW2B")
        E2_0 = wts.tile([97, 144], F32R, tag="E2_0")
        E2_1 = wts.tile([96, 144], F32R, tag="E2_1")
        E2_2 = wts.tile([96, 144], F32R, tag="E2_2")
        E3_0 = wts.tile([73, 1000], F32R, tag="E3_0")
        E3_1 = wts.tile([72, 1000], F32R, tag="E3_1")
        IDENT = wts.tile([64, 64], F32, tag="IDENT")
        C1B = wts.tile([64, 1], F32, tag="C1B")
        C2B = wts.tile([128, 2], F32, tag="C2B")
        C3B = wts.tile([128, 3], F32, tag="C3B")
        C4B = wts.tile([128, 2], F32, tag="C4B")
        C5B = wts.tile([128, 2], F32, tag="C5B")
        GB1 = wts.tile([72, 1], F32, tag="GB1")
        EB1 = wts.tile([96, 3], F32, tag="EB1")
        ESEL = wts.tile([64, 8], F32, tag="ESEL")

        for t, a in [(W1, io["w1n"]), (W2A, io["w2a"]), (W2B, io["w2b"]),
                     (W3A, io["w3a"]), (W3B, io["w3b"]),
                     (GW2B, io["gw2b"]), (IDENT, io["ident"]),
                     (C1B, io["c1bp"]), (C2B, io["c2bp"]), (C3B, io["c3bp"]),
                     (C4B, io["c4bp"]), (C5B, io["c5bp"]), (GB1, io["gb1p"]),
                     (EB1, io["eb1p"]), (ESEL, io["esel"])]:
            nc.scalar.dma_start(t[:], a)
        nc.scalar.dma_start(E2_0[:], io["e2r"][0:97, :])
        nc.scalar.dma_start(E2_1[:], io["e2r"][97:193, :])
        nc.scalar.dma_start(E2_2[:], io["e2r"][193:289, :])
        nc.scalar.dma_start(E3_0[:], io["e3r"][0:73, :])
        nc.scalar.dma_start(E3_1[:], io["e3r"][73:145, :])

        # ---- persistent padded activation tiles (borders stay zero)
        P1D = [acts.tile([128, 2, 31, 32], F32R, tag=f"P1D{k}", name=f"P1D{k}")
               for k in range(1)]
        P2 = [(acts.tile([128, 2, 15, 16], F32R, tag=f"P2a{k}", name=f"P2a{k}"),
               acts.tile([64, 2, 15, 16], F32R, tag=f"P2b{k}", name=f"P2b{k}"))
              for k in range(2)]
        P3 = [[acts.tile([128, 2, 15, 16], F32R, tag=f"P3_{k}_{m}", name=f"P3_{k}_{m}")
               for m in range(3)] for k in range(1)]
        P4 = [[acts.tile([128, 2, 15, 16], F32R, tag=f"P4_{k}_{m}", name=f"P4_{k}_{m}")
               for m in range(2)] for k in range(1)]
        def zfill(t):
            f = t[:].rearrange("c a h w -> c (a h w)")
            nc.gpsimd.dma_start(f, io["zeros"][0:f.shape[0], 0:f.shape[1]])
        for t in P1D:
            zfill(t)
        for k in range(2):
            zfill(P2[k][0])
            zfill(P2[k][1])
        for m in range(3):
            zfill(P3[0][m])
        for m in range(2):
            zfill(P4[0][m])

        # DRAM bounce buffers for collectives
        feat_loc = dram.tile([IPC, 256, 36], F32R)
        feat_all = dram.tile([64, 256, 36], F32R,
                             addr_space="Local" if single else "Shared")
        ar_in = dram.tile([64, 1000], F32)
        ar_out = dram.tile([64, 1000], F32,
                           addr_space="Local" if single else "Shared")

        with (
            tc.tile_pool(name="c1ps", bufs=2, space="PSUM") as c1ps,
            tc.tile_pool(name="c2ps", bufs=1, space="PSUM") as c2ps,
            tc.tile_pool(name="c345ps", bufs=4, space="PSUM") as c345ps,
        ):
            for p in range(PAIRS):
                _conv_trunk_pair(nc, tc, io, p, xs, x36p, P1D, P2, P3, P4,
                                 W1, W2A, W2B, W3A, W3B, w45p,
                                 C1B, C2B, C3B, C4B, C5B,
                                 acts, pl2, c1ps, c2ps, c345ps, feat_loc)

        # ---- AllGather features
        if single:
            for r in range(NCORES):
                nc.sync.dma_start(feat_all[8 * r:8 * (r + 1), :, :], feat_loc[:])
        else:
            nc.gpsimd.collective_compute(
                "AllGather", ALU.bypass,
                replica_groups=[list(range(NCORES))],
                ins=[feat_loc[:].opt()],
                outs=[feat_all[:].opt()],
            )

        with tc.tile_pool(name="mps", bufs=1, space="PSUM") as mps:
            _moe(nc, tc, io, feat_all, gwp, ew1p, wts, acts, mps,
                 GW2B, GB1, EB1, E2_0, E2_1, E2_2, E3_0, E3_1, IDENT, ESEL,
                 ar_in)

        # ---- AllReduce the gate-weighted expert outputs
        if single:
            nc.sync.dma_start(ar_out[:], ar_in[:])
        else:
            nc.gpsimd.collective_compute(
                "AllReduce", ALU.add,
                replica_groups=[list(range(NCORES))],
                ins=[ar_in[:].opt()],
                outs=[ar_out[:].opt()],
            )
        nc.sync.dma_start(io["out"], ar_out[:])


def _conv_trunk_pair(nc, tc, io, p, xs, x36p, P1D, P2, P3, P4,
                     W1, W2A, W2B, W3A, W3B, w45p,
                     C1B, C2B, C3B, C4B, C5B,
                     acts, pl2, c1ps, c2ps, c345ps, feat_loc):
    k = p % 2  # parity for double-buffered padded tiles
    P1d = P1D[0]
    P2a, P2b = P2[k]

    # ---- conv1: X36[(dy,KH,c), y', v] = xpad[c, 4*(y0+y'+KH)+dy-2, v] loaded
    # per row-chunk (contiguous-row DMAs); 11 kw taps read with stride-4 APs.
    row_chunks = [(0, 14), (14, 14), (28, 14), (42, 13)]
    for j in range(2):
        A1 = acts.tile([64, 55, 55], F32R, tag="A1", name="A1")
        for ci, (y0, rows) in enumerate(row_chunks):
            xt = x36p.tile([36, 14, 232], F32R, tag="X36", name="X36")
            if ci in (0, 1, 3):
                f = xt[:].rearrange("c a v -> c (a v)")
                nc.gpsimd.dma_start(f, io["zeros"][0:36, 0:f.shape[1]])
            for dy in range(4):
                for KH in range(3):
                    pb = (dy * 3 + KH) * 3
                    eng = (nc.sync, nc.scalar, nc.gpsimd)[(dy * 3 + KH) % 3]
                    ylo = 0
                    while 4 * (y0 + ylo + KH) + dy - 2 < 0:
                        ylo += 1
                    yhi = rows
                    while yhi > ylo and 4 * (y0 + yhi - 1 + KH) + dy - 2 > 223:
                        yhi -= 1
                    if yhi <= ylo:
                        continue
                    r0 = 4 * (y0 + ylo + KH) + dy - 2
                    nstep = yhi - ylo
                    eng.dma_start(
                        xt[pb:pb + 3, ylo:yhi, 2:226],
                        xs[2 * p + j, :, r0:r0 + 4 * (nstep - 1) + 1:4, :])
            for (s0, sr) in [(0, 7), (7, rows - 7)]:
                ps = c1ps.tile([64, 7, 56], F32, tag="c1")
                for kw in range(11):
                    nc.tensor.matmul(
                        ps[:, 0:sr, :],
                        W1[:, kw, :],
                        xt[:, s0:s0 + sr, kw:kw + 221:4],
                        start=(kw == 0), stop=(kw == 10))
                nc.scalar.activation(
                    A1[:, y0 + s0:y0 + s0 + sr, :], ps[:, 0:sr, 0:55],
                    AF.Relu, bias=C1B[:])
        # pool1: 55x55 -> 27x27 into P1d interior (pad=2)
        nc.vector.tensor_reduce(
            out=P1d[0:64, j, 2:29, 2:29],
            in_=win3s2(A1[:].bitcast(F32), 27, 27, 55),
            axis=AX.XY, op=ALU.max)
    # replicate row-shifted copy into upper partitions: P1d[64+c, j, h] = P1d[c, j, h+1]
    for j in range(2):
        nc.sync.dma_start(P1D[0][64:128, j, 0:30, :], P1D[0][0:64, j, 1:31, :])

    # ---- conv2 (kh-folded: khp0/1 K=128, khp2 K=64; 5 kw; M=192) -> A2, pool2 -> P2
    for j in range(2):
        A2a = acts.tile([128, 27, 27], F32R, tag="A2a", name="A2a")
        A2b = acts.tile([64, 27, 27], F32R, tag="A2b", name="A2b")
        for rc, (y0, rows) in enumerate([(0, 14), (14, 13)]):
            for mc, (msz, A2, ps_tag) in enumerate(
                    [(128, A2a, "c2a"), (64, A2b, "c2b")]):
                mlo = mc * 128
                ps = c2ps.tile([msz, 14, 28], F32, tag=ps_tag)
                first = True
                for khp in range(2):
                    for kw in range(5):
                        nc.tensor.matmul(
                            ps[:, 0:rows, :],
                            W2A[:, khp, kw, mlo:mlo + msz],
                            P1d[:, j, y0 + 2 * khp:y0 + 2 * khp + rows, kw:kw + 28],
                            start=first, stop=False)
                        first = False
                for kw in range(5):
                    nc.tensor.matmul(
                        ps[:, 0:rows, :],
                        W2B[:, kw, mlo:mlo + msz],
                        P1d[0:64, j, y0 + 4:y0 + 4 + rows, kw:kw + 28],
                        start=False, stop=(kw == 4))
                nc.scalar.activation(A2[:, y0:y0 + rows, :], ps[:, 0:rows, 0:27],
                                     AF.Relu, bias=C2B[0:msz, mc:mc + 1])
        # pool2: 27x27 -> 13x13 into P2 interior (pad=1)
        nc.vector.tensor_reduce(
            out=P2a[:, j, 1:14, 1:14],
            in_=win3s2(A2a[:].bitcast(F32), 13, 13, 27),
            axis=AX.XY, op=ALU.max)
        nc.vector.tensor_reduce(
            out=P2b[:, j, 1:14, 1:14],
            in_=win3s2(A2b[:].bitcast(F32), 13, 13, 27),
            axis=AX.XY, op=ALU.max)

    P3k, P4k = P3[0], P4[0]

    # ---- conv3: K=192 (128+64), M=384 (3x128), both images at once
    for mc in range(3):
        ps = c345ps.tile([128, 2, 13, 14], F32, tag="c345")
        first = True
        for kh in range(3):
            for kw in range(3):
                nc.tensor.matmul(
                    ps[:], W3A[:, kh * 3 + kw, mc * 128:(mc + 1) * 128],
                    P2a[:, :, kh:kh + 13, kw:kw + 14],
                    start=first, stop=False)
                first = False
                nc.tensor.matmul(
                    ps[:], W3B[:, kh * 3 + kw, mc * 128:(mc + 1) * 128],
                    P2b[:, :, kh:kh + 13, kw:kw + 14],
                    start=False, stop=(kh == 2 and kw == 2))
        nc.scalar.activation(P3k[mc][:, :, 1:14, 1:14], ps[:, :, :, 0:13],
                             AF.Relu, bias=C3B[:, mc:mc + 1])

    # ---- conv4: K=384 (3x128), M=256 (2x128), weights streamed per kc
    ps4 = [c345ps.tile([128, 2, 13, 14], F32, tag="c345", name=f"c4ps{m}")
           for m in range(2)]
    first = True
    for kc in range(3):
        w4s = w45p.tile([128, 9, 256], F32R, tag="w45s", name="w4s")
        nc.scalar.dma_start(w4s[:], io["w4t"][:, kc, :, :])
        for kh in range(3):
            for kw in range(3):
                for mc in range(2):
                    nc.tensor.matmul(
                        ps4[mc][:], w4s[:, kh * 3 + kw, mc * 128:(mc + 1) * 128],
                        P3k[kc][:, :, kh:kh + 13, kw:kw + 14],
                        start=first, stop=(kh == 2 and kw == 2 and kc == 2))
                first = False
    for mc in range(2):
        nc.scalar.activation(P4k[mc][:, :, 1:14, 1:14], ps4[mc][:, :, :, 0:13],
                             AF.Relu, bias=C4B[:, mc:mc + 1])

    # ---- conv5: K=256 (2x128), M=256 (2x128), streamed weights -> A5, pool3 -> F
    ps5 = [c345ps.tile([128, 2, 13, 14], F32, tag="c345", name=f"c5ps{m}")
           for m in range(2)]
    first = True
    for kc in range(2):
        w5s = w45p.tile([128, 9, 256], F32R, tag="w45s", name="w5s")
        nc.scalar.dma_start(w5s[:], io["w5t"][:, kc, :, :])
        for kh in range(3):
            for kw in range(3):
                for mc in range(2):
                    nc.tensor.matmul(
                        ps5[mc][:], w5s[:, kh * 3 + kw, mc * 128:(mc + 1) * 128],
                        P4k[kc][:, :, kh:kh + 13, kw:kw + 14],
                        start=first, stop=(kh == 2 and kw == 2 and kc == 1))
                first = False
    for mc in range(2):
        A5 = pl2.tile([128, 2, 13, 13], F32R, tag=f"A5_{mc}", name=f"A5_{mc}")
        nc.scalar.activation(A5[:], ps5[mc][:, :, :, 0:13], AF.Relu,
                             bias=C5B[:, mc:mc + 1])
        F = pl2.tile([128, 2, 6, 6], F32R, tag=f"F_{mc}", name=f"F_{mc}")
        for j in range(2):
            nc.vector.tensor_reduce(
                out=F[:, j],
                in_=win3s2(A5[:, j].bitcast(F32), 6, 6, 13),
                axis=AX.XY, op=ALU.max)
        # store features channel-major: feat_loc[b, c, s]
        nc.sync.dma_start(
            feat_loc[:].rearrange("b c s -> c b s")
            [mc * 128:(mc + 1) * 128, 2 * p:2 * p + 2, :],
            F[:].rearrange("c j x y -> c j (x y)"))


def _moe(nc, tc, io, feat_all, gwp, ew1p, wts, acts, mps,
         GW2B, GB1, EB1, E2_0, E2_1, E2_2, E3_0, E3_1, IDENT, ESEL, ar_in):
    # load gathered features channel-major: FA[kc] = [128, 64img, 36]
    FA = [acts.tile([128, 64, 36], F32R, tag=f"FA{kc}", name=f"FA{kc}")
          for kc in range(2)]
    for kc in range(2):
        nc.sync.dma_start(
            FA[kc][:],
            feat_all[:].rearrange("b c s -> c b s")[kc * 128:(kc + 1) * 128, :, :])

    # ---- gate: g = relu(feat @ gw1 + gb1) (g kept transposed [72, 64])
    gps = mps.tile([72, 64], F32, tag="gps")
    first = True
    for kc in range(2):
        for sb in range(3):
            gt = gwp.tile([128, 12, 72], F32R, tag="gw1", name="gt")
            nc.scalar.dma_start(gt[:], io["gw1r"][:, kc, sb * 12:(sb + 1) * 12, :])
            for si in range(12):
                s = sb * 12 + si
                nc.tensor.matmul(gps[:], gt[:, si, :], FA[kc][:, :, s],
                                 start=first, stop=(kc == 1 and s == 35))
                first = False
    G1 = acts.tile([73, 64], F32R, tag="G1")
    nc.scalar.activation(G1[0:72, :], gps[:], AF.Relu, bias=GB1[:])
    nc.sync.dma_start(G1[72:73, :], io["ones64"])
    # logits[64, 8] = G1.T @ [gw2; gb2]
    lps = mps.tile([64, 8], F32, tag="lps")
    nc.tensor.matmul(lps[:], G1[:], GW2B[:], start=True, stop=True)
    L = acts.tile([64, 8], F32, tag="L")
    nc.scalar.activation(L[:], lps[:], AF.Copy)

    # ---- top-2 softmax -> this core's gate column [64, 1]
    v1 = acts.tile([64, 1], F32, tag="v1")
    nc.vector.tensor_reduce(out=v1[:], in_=L[:], axis=AX.X, op=ALU.max)
    nv1 = acts.tile([64, 1], F32, tag="nv1")
    nc.vector.tensor_scalar_mul(nv1[:], v1[:], -1.0)
    m1 = acts.tile([64, 8], F32, tag="m1")
    nc.vector.tensor_scalar(out=m1[:], in0=L[:], scalar1=v1[:], scalar2=None,
                            op0=ALU.is_equal)
    lm = acts.tile([64, 8], F32, tag="lm")
    nc.vector.scalar_tensor_tensor(out=lm[:], in0=m1[:], scalar=-1e30,
                                   in1=L[:], op0=ALU.mult, op1=ALU.add)
    v2 = acts.tile([64, 1], F32, tag="v2")
    nc.vector.tensor_reduce(out=v2[:], in_=lm[:], axis=AX.X, op=ALU.max)
    e2v = acts.tile([64, 1], F32, tag="e2v")
    nc.scalar.activation(e2v[:], v2[:], AF.Exp, bias=nv1[:])
    den = acts.tile([64, 1], F32, tag="den")
    nc.vector.tensor_scalar_add(den[:], e2v[:], 1.0)
    inv = acts.tile([64, 1], F32, tag="inv")
    nc.vector.reciprocal(inv[:], den[:])
    expl = acts.tile([64, 8], F32, tag="expl")
    nc.scalar.activation(expl[:], L[:], AF.Exp, bias=nv1[:])
    msk = acts.tile([64, 8], F32, tag="msk")
    nc.vector.tensor_scalar(out=msk[:], in0=L[:], scalar1=v2[:], scalar2=None,
                            op0=ALU.is_ge)
    t1 = acts.tile([64, 8], F32, tag="t1")
    nc.vector.tensor_tensor(out=t1[:], in0=msk[:], in1=expl[:], op=ALU.mult)
    t2 = acts.tile([64, 8], F32, tag="t2")
    nc.vector.tensor_tensor(out=t2[:], in0=t1[:], in1=ESEL[:], op=ALU.mult)
    gs = acts.tile([64, 1], F32, tag="gs")
    nc.vector.tensor_reduce(out=gs[:], in_=t2[:], axis=AX.X, op=ALU.add)
    gate = acts.tile([64, 1], F32, tag="gate")
    nc.vector.tensor_scalar(out=gate[:], in0=gs[:], scalar1=inv[:], scalar2=None,
                            op0=ALU.mult)

    # ---- FC1: h1T[mc] = relu(sum_{kc,s} ew1[kc,s].T @ featT + eb1) ([96, 64] x3)
    h1ps = [mps.tile([96, 64], F32, tag=f"h1ps{m}", name=f"h1ps{m}")
            for m in range(3)]
    first = True
    for kc in range(2):
        for sb in range(6):
            et = ew1p.tile([128, 6, 288], F32R, tag="ew1", name="et")
            nc.scalar.dma_start(et[:], io["ew1r"][:, kc, sb * 6:(sb + 1) * 6, :])
            for si in range(6):
                s = sb * 6 + si
                for m in range(3):
                    nc.tensor.matmul(h1ps[m][:], et[:, si, m * 96:(m + 1) * 96],
                                     FA[kc][:, :, s],
                                     start=first, stop=(kc == 1 and s == 35))
                first = False
    H1 = [acts.tile([97 if m == 0 else 96, 64], F32R, tag=f"H1_{m}", name=f"H1_{m}")
          for m in range(3)]
    for m in range(3):
        nc.scalar.activation(H1[m][0:96, :], h1ps[m][:], AF.Relu,
                             bias=EB1[:, m:m + 1])
    nc.sync.dma_start(H1[0][96:97, :], io["ones64"])

    # ---- FC2: h2[64, 144] = relu(h1 @ ew2 + eb2)
    h2ps = mps.tile([64, 144], F32, tag="h2ps")
    nc.tensor.matmul(h2ps[:], H1[0][:], E2_0[:], start=True, stop=False)
    nc.tensor.matmul(h2ps[:], H1[1][:], E2_1[:], start=False, stop=False)
    nc.tensor.matmul(h2ps[:], H1[2][:], E2_2[:], start=False, stop=True)
    H2 = acts.tile([64, 144], F32, tag="H2")
    nc.scalar.activation(H2[:], h2ps[:], AF.Relu)

    # transpose h2 -> [72, 64] x2 (PE transpose via identity)
    H2T = [acts.tile([73 if c == 0 else 72, 64], F32R, tag=f"H2T{c}", name=f"H2T{c}")
           for c in range(2)]
    for c in range(2):
        tps = mps.tile([72, 64], F32, tag="tps")
        nc.tensor.transpose(tps[:], H2[:, c * 72:(c + 1) * 72], IDENT[:])
        nc.scalar.activation(H2T[c][0:72, :], tps[:], AF.Copy)
    nc.sync.dma_start(H2T[0][72:73, :], io["ones64"])

    # ---- FC3 + gate scale: eo[64, 1000] = (h2 @ ew3 + eb3) * gate
    for nchunk in range(2):
        nlo = nchunk * 500
        eps = mps.tile([64, 500], F32, tag="eps", name="eps")
        nc.tensor.matmul(eps[:], H2T[0][:], E3_0[:, nlo:nlo + 500],
                         start=True, stop=False)
        nc.tensor.matmul(eps[:], H2T[1][:], E3_1[:, nlo:nlo + 500],
                         start=False, stop=True)
        eps_s = acts.tile([64, 500], F32, tag="eps_s", name="eps_s")
        nc.scalar.activation(eps_s[:], eps[:], AF.Copy, scale=gate[:])
        nc.sync.dma_start(ar_in[:, nlo:nlo + 500], eps_s[:])


# ---------------------------------------------------------------- runner

_CACHE = {}


def _get_state():
    if "nc" not in _CACHE:
        _CACHE["nc"] = build_nc()
    return _CACHE["nc"]


def _get_runner():
    """Cached jitted SPMD executor (device mesh over 8 cores, no donation)."""
    if "runner" in _CACHE:
        return _CACHE["runner"]
    import jax
    from jax.sharding import Mesh, PartitionSpec
    from jax.experimental.shard_map import shard_map
    from concourse.bass2jax import (_bass_exec_p, install_neuronx_cc_hook,
                                    partition_id_tensor)

    nc = _get_state()
    install_neuronx_cc_hook()
    partition_name = nc.partition_id_tensor.name if nc.partition_id_tensor else None
    in_names, out_names, out_avals, zero_outs = [], [], [], []
    for alloc in nc.m.functions[0].allocations:
        if not isinstance(alloc, mybir.MemoryLocationSet):
            continue
        name = alloc.memorylocations[0].name
        if alloc.kind == "ExternalInput":
            if name != partition_name:
                in_names.append(name)
        elif alloc.kind == "ExternalOutput":
            shape = tuple(alloc.tensor_shape)
            dtype = mybir.dt.np(alloc.dtype)
            out_names.append(name)
            out_avals.append(jax.core.ShapedArray(shape, dtype))
            zero_outs.append(np.zeros(shape, dtype))
    all_in = in_names + out_names + ([partition_name] if partition_name else [])

    def _body(*args):
        operands = list(args)
        if partition_name is not None:
            operands.append(partition_id_tensor())
        return tuple(_bass_exec_p.bind(
            *operands, out_avals=tuple(out_avals), in_names=tuple(all_in),
            out_names=tuple(out_names), lowering_input_output_aliases=(),
            sim_require_finite=True, sim_require_nnan=True, nc=nc))

    devices = jax.devices()[:NCORES]
    mesh = Mesh(np.asarray(devices), ("core",))
    nin = len(in_names) + len(out_names)
    fn = jax.jit(shard_map(_body, mesh=mesh,
                           in_specs=(PartitionSpec("core"),) * nin,
                           out_specs=(PartitionSpec("core"),) * len(out_names),
                           check_rep=False), keep_unused=True)
    _CACHE["runner"] = (fn, in_names, out_names, zero_outs)
    return _CACHE["runner"]


def _make_in_maps(inputs):
    static = _prep_static(
        inputs["c1w"], inputs["c2w"], inputs["c3w"], inputs["c4w"],
        inputs["c5w"], inputs["gw1"], inputs["gw2"], inputs["gb2"],
        inputs["c1b"], inputs["c2b"], inputs["c3b"], inputs["c4b"],
        inputs["c5b"], inputs["gb1"])
    x = np.asarray(inputs["x"], np.float32)
    in_maps = []
    for r in range(NCORES):
        m = dict(static)
        m.update(_prep_expert(inputs["ew1"], inputs["eb1"], inputs["ew2"],
                              inputs["eb2"], inputs["ew3"], inputs["eb3"], r))
        m["xs"] = np.ascontiguousarray(x[IPC * r:IPC * (r + 1)])
        in_maps.append({k: np.asarray(v) for k, v in m.items()})
    return in_maps


def kernel(**inputs):
    inputs = {k: np.asarray(v) for k, v in inputs.items()}
    fn, in_names, out_names, zero_outs = _get_runner()
    in_maps = _make_in_maps(inputs)
    concat_in = [np.concatenate([np.asarray(in_maps[c][n])
                                 for c in range(NCORES)], axis=0)
                 for n in in_names]
    concat_zero = [np.zeros((NCORES * z.shape[0], *z.shape[1:]), z.dtype)
                   for z in zero_outs]
    outs = fn(*concat_in, *concat_zero)
    oi = out_names.index("out")
    full = np.asarray(outs[oi]).reshape(NCORES, 64, 1000)
    return full[0].astype(np.float32)
